# revision 9
# baseline (speedup 1.0000x reference)
"""Multi-head attention (B=2, N=2048, C=768, H=12) on 8 trn2 cores.

Sharding: core i handles batch b = i//4 and head-group g = i%4 (3 heads each).
All matmul operands are bf16 (fast LDWEIGHTS, 1024-wide moving operands);
PSUM accumulation and the softmax normalization chain stay fp32.

Per-core pipeline:
  1. QKV^T projection from host-pre-transposed bf16 xT [C, N]:
       qT/kT d-major [64, N] per head (heads 0,1 packed in [128, N] tiles,
       head 2's q/k packed together, k2 extracted via SBUF-SBUF DMA);
       v n-major [N, 64] per head augmented with a ones column
       (softmax-denominator trick).
  2. Scores transposed: S^T[k, q] = kT-slice.T @ qT (contraction d=64), exp
       via ScalarE with fused 1/sqrt(d) scale, output bf16.
  3. attn@V with lhsT = [1 | 0 | v]: out row 0 = denominators, rows 64:128 =
       unnormalized attn_out^T; accumulated over k chunks in fp32 PSUM.
  4. Normalize: DVE reciprocal of row 0, gpsimd partition-broadcast, DVE
       multiply -> bf16 attn_out^T shard [192, QW].
  5. Per-window AllGather (bf16) within same-batch groups [[0-3],[4-7]]
       -> [768, QW]; window 0's gather overlaps window 1's attention.
  6. Output projection w_proj column-shard (contraction 768, no zero pad),
       bias added as per-partition DVE scalar; out^T [192, N] fp32.
"""

import numpy as np

B, N, C, H, HD = 2, 2048, 768, 12, 64
G = 4              # tensor-parallel head groups
HL = H // G        # 3 heads per core
CHL = HL * HD      # 192 local channels
SCALE = HD ** -0.5
NCORES = 8
CT = C // 128      # 6 contraction chunks
NT = N // 128      # 16 n chunks (= k chunks)
QW = 1024          # q window width
NWIN = N // QW     # 2 windows
KT = N // 128      # 16 k chunks
FW = 512           # fp32-out matmul free width (psum bank)
SW = 512           # matmul moving-operand width limit
KP = C // 128      # 6 proj contraction chunks

_CACHE = {}


def _build_nc():
    import concourse.bass as bass
    import concourse.bacc as bacc
    import concourse.tile as tile
    import concourse.mybir as mybir

    F32 = mybir.dt.float32
    BF16 = mybir.dt.bfloat16
    AF = mybir.ActivationFunctionType

    nc = bacc.Bacc(num_devices=NCORES)
    xT_d = nc.declare_dram_parameter("xT", [C, N], BF16, isOutput=False)
    # wqk blocks: [0:128] = q heads 0,1; [128:256] = k heads 0,1;
    # [256:320] = q head 2; [320:384] = k head 2
    wqk_d = nc.declare_dram_parameter("wqk", [C, 384], BF16, isOutput=False)
    wv_d = nc.declare_dram_parameter("wv", [C, CHL], BF16, isOutput=False)
    wp_d = nc.declare_dram_parameter("wp", [C, CHL], BF16, isOutput=False)
    bp_d = nc.declare_dram_parameter("bp", [CHL, 1], F32, isOutput=False)
    out_d = nc.declare_dram_parameter("out", [CHL, N], F32, isOutput=True)

    with tile.TileContext(nc) as tc:
        with tc.tile_pool(name="dram", bufs=1, space="DRAM") as dram:
            ag_ins = [dram.tile([CHL, QW], BF16, name=f"ag_in{w}")
                      for w in range(NWIN)]
            ag_outs = [dram.tile([G * CHL, QW], BF16, name=f"ag_out{w}")
                       for w in range(NWIN)]

            with tc.tile_pool(name="persist", bufs=1) as P:
                # ---- inputs only needed through phase 1 (own pool) ----
                QIN = tc.alloc_tile_pool(name="qkv_in", bufs=1)
                xT_sb = QIN.tile([128, CT, N], BF16)
                wqk_sb = QIN.tile([128, CT, 384], BF16)
                wv_sb = QIN.tile([128, CT, CHL], BF16)
                for ct in range(CT):
                    rs = slice(ct * 128, (ct + 1) * 128)
                    nc.sync.dma_start(out=wqk_sb[:, ct, :], in_=wqk_d[rs, :])
                    nc.sync.dma_start(out=wv_sb[:, ct, :], in_=wv_d[rs, :])
                # split xT over many DMA queues
                XS = 4
                for ct in range(CT):
                    rs = slice(ct * 128, (ct + 1) * 128)
                    for xs in range(XS):
                        cs = slice(xs * (N // XS), (xs + 1) * (N // XS))
                        nc.sync.dma_start(out=xT_sb[:, ct, cs],
                                          in_=xT_d[rs, cs])
                wp_sb = P.tile([128, KP, CHL], BF16)
                for kp in range(KP):
                    nc.sync.dma_start(
                        out=wp_sb[:, kp, :],
                        in_=wp_d[kp * 128:(kp + 1) * 128, :],
                    )
                bpa_sb = P.tile([128, 1], F32)
                bpb_sb = P.tile([64, 1], F32)
                nc.sync.dma_start(out=bpa_sb[:], in_=bp_d[0:128, :])
                nc.sync.dma_start(out=bpb_sb[:], in_=bp_d[128:CHL, :])

                # ---- persistent QKV results (bf16) ----
                q01_sb = P.tile([128, N], BF16)   # qT heads 0,1
                k01_sb = P.tile([128, N], BF16)
                qk2_sb = P.tile([128, N], BF16)   # rows 0:64 q2, 64:128 k2
                k2_sb = P.tile([64, N], BF16)     # k2 re-based to partition 0
                # [n, nt, h, 128]: col 0 = ones (sums), 1:64 zero, 64:128 = v
                v_sb = P.tile([128, NT, HL, 2 * HD], BF16)
                nc.vector.memset(v_sb[:, :, :, 0:1], 1.0)
                nc.vector.memset(v_sb[:, :, :, 1:HD], 0.0)

                # ---- phase 1: QKV projections ----
                with tc.tile_pool(name="qkv_ps", bufs=1, space="PSUM") as QP:
                    # warm-up matmuls: keep the PE busy during the input DMA
                    # so the HAM clock gate opens before real work arrives.
                    warm_sb = P.tile([128, FW], BF16)
                    nc.vector.memset(warm_sb[:], 0.0)
                    for _ in range(10):
                        wm = QP.tile([128, FW], F32, tag="qk", bufs=4)
                        for r in range(CT):
                            nc.tensor.matmul(
                                wm[:], lhsT=warm_sb[:, 0:128], rhs=warm_sb[:],
                                start=(r == 0), stop=(r == CT - 1),
                            )
                    for dst, blk in ((k01_sb, 1), (q01_sb, 0), (qk2_sb, 2)):
                        for f in range(N // FW):
                            ps = QP.tile([128, FW], F32, tag="qk", bufs=4)
                            for ct in range(CT):
                                nc.tensor.matmul(
                                    ps[:],
                                    lhsT=wqk_sb[:, ct,
                                                blk * 128:(blk + 1) * 128],
                                    rhs=xT_sb[:, ct, f * FW:(f + 1) * FW],
                                    start=(ct == 0), stop=(ct == CT - 1),
                                )
                            nc.vector.tensor_copy(
                                dst[:, f * FW:(f + 1) * FW], ps[:])
                    # k2 shard -> its own partition-0-based tile
                    nc.sync.dma_start(out=k2_sb[:], in_=qk2_sb[64:128, :])
                    for nt in range(NT):
                        v_ps = QP.tile([128, CHL], F32, tag="v", bufs=2)
                        for ct in range(CT):
                            nc.tensor.matmul(
                                v_ps[:],
                                lhsT=xT_sb[:, ct, nt * 128:(nt + 1) * 128],
                                rhs=wv_sb[:, ct, :],
                                start=(ct == 0), stop=(ct == CT - 1),
                            )
                        nc.vector.tensor_copy(
                            v_sb[:, nt, :, HD:2 * HD],
                            v_ps[:].rearrange("p (h d) -> p h d", h=HL))
                QIN.release()

                # ---- phases 2-4: attention + gather + projection ----
                # One PSUM pool so window-0 projection overlaps window-1
                # gather: A(2) + S(2x2) + pr(2) = 8 banks.
                with tc.tile_pool(name="att_ps", bufs=1, space="PSUM") as AT, \
                        tc.tile_pool(name="att_sb", bufs=1) as AS:
                    heads = ((q01_sb[0:64], k01_sb[0:64]),
                            (q01_sb[64:128], k01_sb[64:128]),
                            (qk2_sb[0:64], k2_sb[0:64]))
                    for w in range(NWIN):
                        q0 = w * QW
                        for h in range(HL):
                            qh, kh = heads[h]
                            A = AT.tile([128, QW], F32, tag="A", bufs=1)
                            # software-pipelined: emit S(kc+1) before AV(kc)
                            # so the in-order tensor queue never blocks on
                            # exp(kc) before issuing the next scores matmul.
                            Es = []
                            for kc in range(KT):
                                S = AT.tile([128, QW], F32, tag="S", bufs=2)
                                E = AS.tile([128, QW], BF16, tag="E", bufs=4)
                                for j in range(QW // SW):
                                    js = slice(j * SW, (j + 1) * SW)
                                    nc.tensor.matmul(
                                        S[:, js],
                                        lhsT=kh[:, kc * 128:(kc + 1) * 128],
                                        rhs=qh[:, q0 + j * SW:q0 + (j + 1) * SW],
                                    )
                                nc.scalar.activation(E[:], S[:], AF.Exp,
                                                     scale=SCALE)
                                Es.append((kc, E))
                                if len(Es) == 2:
                                    pk, pE = Es.pop(0)
                                    for j in range(QW // SW):
                                        js = slice(j * SW, (j + 1) * SW)
                                        nc.tensor.matmul(
                                            A[:, js],
                                            lhsT=v_sb[:, pk, h, :],
                                            rhs=pE[:, js],
                                            start=(pk == 0), stop=False,
                                        )
                            pk, pE = Es.pop(0)
                            for j in range(QW // SW):
                                js = slice(j * SW, (j + 1) * SW)
                                nc.tensor.matmul(
                                    A[:, js],
                                    lhsT=v_sb[:, pk, h, :],
                                    rhs=pE[:, js],
                                    start=False, stop=True,
                                )
                            # normalize rows 64:128 by 1/row0. The DVE works
                            # each partition serially, so a [1,QW] reciprocal
                            # is single-lane (~6.5us); reshape the row across
                            # 64 partitions via SBUF-SBUF DMA instead.
                            d0 = AS.tile([1, QW], F32, tag="d0", bufs=2)
                            dT = AS.tile([64, QW // 64], F32, tag="dT",
                                         bufs=2)
                            rT = AS.tile([64, QW // 64], F32, tag="rT",
                                         bufs=2)
                            r0 = AS.tile([1, QW], F32, tag="r0", bufs=2)
                            bcs = AS.tile([128, QW], F32, tag="bcs", bufs=2)
                            attn_t = AS.tile([128, QW], BF16, tag="attn",
                                             bufs=3)
                            nc.vector.tensor_copy(d0[:], A[0:1, :])
                            nc.sync.dma_start(out=dT[:], in_=d0[:])
                            nc.vector.reciprocal(rT[:], dT[:])
                            nc.sync.dma_start(out=r0[:], in_=rT[:])
                            nc.gpsimd.partition_broadcast(bcs[:], r0[0:1, :])
                            nc.vector.tensor_mul(attn_t[64:128, :],
                                                 A[64:128, :],
                                                 bcs[64:128, :])
                            nc.sync.dma_start(
                                out=ag_ins[w][h * HD:(h + 1) * HD, :],
                                in_=attn_t[64:128, :],
                            )
                        # same-batch 4-core AllGather; window 0's gather
                        # overlaps window 1's attention compute
                        nc.gpsimd.collective_compute(
                            "AllGather",
                            mybir.AluOpType.bypass,
                            replica_groups=[[0, 1, 2, 3], [4, 5, 6, 7]],
                            ins=[ag_ins[w].opt()],
                            outs=[ag_outs[w].opt()],
                        )

                    # ---- output projection (out^T [CHL, N]) ----
                    for f in range(N // FW):
                        wf, jf = divmod(f, QW // FW)
                        ao = AS.tile([128, KP, FW], BF16, tag="ao", bufs=2)
                        for kp in range(KP):
                            nc.sync.dma_start(
                                out=ao[:, kp, :],
                                in_=ag_outs[wf][kp * 128:(kp + 1) * 128,
                                                jf * FW:(jf + 1) * FW],
                            )
                        for mlo, mhi, bp_sb in ((0, 128, bpa_sb),
                                                (128, CHL, bpb_sb)):
                            m = mhi - mlo
                            pr = AT.tile([m, FW], F32, tag="pr", bufs=2,
                                         padded_shape=[128, FW])
                            for kp in range(KP):
                                nc.tensor.matmul(
                                    pr[:],
                                    lhsT=wp_sb[:, kp, mlo:mhi],
                                    rhs=ao[:, kp, :],
                                    start=(kp == 0), stop=(kp == KP - 1),
                                )
                            o_t = AS.tile([m, FW], F32, tag="o", bufs=3,
                                          padded_shape=[128, FW])
                            nc.vector.tensor_scalar_add(o_t[:], pr[:],
                                                        bp_sb[0:m])
                            nc.sync.dma_start(
                                out=out_d[mlo:mhi, f * FW:(f + 1) * FW],
                                in_=o_t[:],
                            )
    nc.finalize()
    return nc


def get_nc():
    if "nc" not in _CACHE:
        _CACHE["nc"] = _build_nc()
    return _CACHE["nc"]


def make_in_maps(x, w_qkv, w_proj, b_proj):
    import ml_dtypes

    BF = ml_dtypes.bfloat16
    x = np.asarray(x, dtype=np.float32)
    w_qkv = np.asarray(w_qkv, dtype=np.float32)
    w_proj = np.asarray(w_proj, dtype=np.float32)
    b_proj = np.asarray(b_proj, dtype=np.float32)
    in_maps = []
    for core in range(NCORES):
        b, g = divmod(core, G)
        cs = slice(g * CHL, (g + 1) * CHL)
        wq = w_qkv[:, 0 * C:1 * C][:, cs]
        wk = w_qkv[:, 1 * C:2 * C][:, cs]
        wqk = np.concatenate(
            [wq[:, 0:128], wk[:, 0:128], wq[:, 128:CHL], wk[:, 128:CHL]],
            axis=1)
        im = {
            "xT": np.ascontiguousarray(x[b].T).astype(BF),
            "wqk": np.ascontiguousarray(wqk).astype(BF),
            "wv": np.ascontiguousarray(
                w_qkv[:, 2 * C:3 * C][:, cs]).astype(BF),
            "wp": np.ascontiguousarray(w_proj[:, cs]).astype(BF),
            "bp": np.ascontiguousarray(b_proj[cs].reshape(CHL, 1)),
        }
        in_maps.append(im)
    return in_maps


def unshard(results):
    out = np.empty((B, N, C), dtype=np.float32)
    for b in range(B):
        outT = np.concatenate(
            [results[b * G + g]["out"] for g in range(G)], axis=0)
        out[b] = outT.T
    return out


def kernel(x, w_qkv, w_proj, b_proj):
    from concourse.bass_utils import run_bass_kernel_spmd

    nc = get_nc()
    in_maps = make_in_maps(x, w_qkv, w_proj, b_proj)
    res = run_bass_kernel_spmd(nc, in_maps, list(range(NCORES)))
    return unshard(res.results)


# revision 11
# speedup vs baseline: 1.3621x; 1.3621x over previous
"""Multi-head attention (B=2, N=2048, C=768, H=12) on 8 trn2 cores.

Sharding: core i handles batch b = i//4 and head-group g = i%4 (3 heads each).
All matmul operands are bf16 (fast LDWEIGHTS, FWL); PSUM accumulation and the
softmax normalization chain stay fp32.

Per-core pipeline:
  1. QKV^T projection from host-pre-transposed bf16 xT [C, N]:
       qT/kT d-major [64, N] per head (heads 0,1 packed in [128, N] tiles,
       head 2's q/k packed together, k2 extracted via SBUF-SBUF DMA);
       v n-major [N, 64] per head augmented with a ones column.
       Emission order: k01, q01(first window), v, [head0 w0 attention],
       qk2, q01(second window), ... so the first exp starts as early as
       possible.
  2. Scores transposed: S^T[k, q] = kT-slice.T @ qT (contraction d=64), exp
       via ScalarE with fused 1/sqrt(d) scale, output bf16. Software
       pipelined: S(kc+1) is emitted before attn@V(kc) so the in-order
       tensor queue never waits on exp.
  3. attn@V with lhsT = [1 | 0 | v]: out row 0 = denominators, rows 64:128 =
       unnormalized attn_out^T; accumulated over k chunks in fp32 PSUM.
  4. Normalize off the critical path: rows copied out of PSUM immediately
       (frees the single accumulator for the next head), denominator row
       reshaped across 64 partitions via SBUF-SBUF DMA (a [1,QW] DVE
       reciprocal would be single-lane), reciprocal, reshape back, gpsimd
       partition-broadcast, DVE multiply -> bf16 attn_out^T shard.
  5. Per-window AllGather (bf16) within same-batch groups [[0-3],[4-7]];
       window 0's gather overlaps window 1's attention.
  6. Output projection w_proj column-shard (contraction 768), bias added as
       per-partition DVE scalar; out^T [192, N] fp32.
"""

import numpy as np

B, N, C, H, HD = 2, 2048, 768, 12, 64
G = 4              # tensor-parallel head groups
HL = H // G        # 3 heads per core
CHL = HL * HD      # 192 local channels
SCALE = HD ** -0.5
NCORES = 8
CT = C // 128      # 6 contraction chunks
NT = N // 128      # 16 n chunks (= k chunks)
QW = 1024          # q window width
NWIN = N // QW     # 2 windows
KT = N // 128      # 16 k chunks
FW = 512           # matmul moving-operand width (psum bank)
KP = C // 128      # 6 proj contraction chunks

_CACHE = {}


def _build_nc():
    import concourse.bass as bass
    import concourse.bacc as bacc
    import concourse.tile as tile
    import concourse.mybir as mybir

    F32 = mybir.dt.float32
    BF16 = mybir.dt.bfloat16
    AF = mybir.ActivationFunctionType

    nc = bacc.Bacc(num_devices=NCORES)
    xT_d = nc.declare_dram_parameter("xT", [C, N], BF16, isOutput=False)
    # wqk blocks: [0:128] = q heads 0,1; [128:256] = k heads 0,1;
    # [256:320] = q head 2; [320:384] = k head 2
    wqk_d = nc.declare_dram_parameter("wqk", [C, 384], BF16, isOutput=False)
    wv_d = nc.declare_dram_parameter("wv", [C, CHL], BF16, isOutput=False)
    wp_d = nc.declare_dram_parameter("wp", [C, CHL], BF16, isOutput=False)
    bp_d = nc.declare_dram_parameter("bp", [CHL, 1], F32, isOutput=False)
    out_d = nc.declare_dram_parameter("out", [CHL, N], F32, isOutput=True)

    with tile.TileContext(nc) as tc:
        with tc.tile_pool(name="dram", bufs=1, space="DRAM") as dram:
            ag_ins = [dram.tile([CHL, QW], BF16, name=f"ag_in{w}")
                      for w in range(NWIN)]
            ag_outs = [dram.tile([G * CHL, QW], BF16, name=f"ag_out{w}")
                       for w in range(NWIN)]

            with tc.tile_pool(name="persist", bufs=1) as P, \
                    tc.tile_pool(name="psum", bufs=1, space="PSUM") as PS, \
                    tc.tile_pool(name="work", bufs=1) as AS:
                # ---- input loads ----
                QIN = tc.alloc_tile_pool(name="qkv_in", bufs=1)
                xT_sb = QIN.tile([128, CT, N], BF16)
                wqk_sb = QIN.tile([128, CT, 384], BF16)
                wv_sb = QIN.tile([128, CT, CHL], BF16)
                for ct in range(CT):
                    rs = slice(ct * 128, (ct + 1) * 128)
                    nc.sync.dma_start(out=wqk_sb[:, ct, :], in_=wqk_d[rs, :])
                    nc.sync.dma_start(out=wv_sb[:, ct, :], in_=wv_d[rs, :])
                XS = 4  # split xT over many DMA queues
                for ct in range(CT):
                    rs = slice(ct * 128, (ct + 1) * 128)
                    for xs in range(XS):
                        cs = slice(xs * (N // XS), (xs + 1) * (N // XS))
                        nc.sync.dma_start(out=xT_sb[:, ct, cs],
                                          in_=xT_d[rs, cs])
                wp_sb = P.tile([128, KP, CHL], BF16)
                for kp in range(KP):
                    nc.sync.dma_start(
                        out=wp_sb[:, kp, :],
                        in_=wp_d[kp * 128:(kp + 1) * 128, :],
                    )
                bpa_sb = P.tile([128, 1], F32)
                bpb_sb = P.tile([64, 1], F32)
                nc.sync.dma_start(out=bpa_sb[:], in_=bp_d[0:128, :])
                nc.sync.dma_start(out=bpb_sb[:], in_=bp_d[128:CHL, :])

                # ---- persistent QKV results (bf16) ----
                q01_sb = P.tile([128, N], BF16)   # qT heads 0,1
                k01_sb = P.tile([128, N], BF16)
                qk2_sb = P.tile([128, N], BF16)   # rows 0:64 q2, 64:128 k2
                k2_sb = P.tile([64, N], BF16)     # k2 re-based to partition 0
                # [n, nt, h, 128]: col 0 = ones (sums), 1:64 zero, 64:128 = v
                v_sb = P.tile([128, NT, HL, 2 * HD], BF16)
                nc.vector.memset(v_sb[:, :, :, 0:1], 1.0)
                nc.vector.memset(v_sb[:, :, :, 1:HD], 0.0)
                warm_sb = P.tile([128, FW], BF16)
                nc.vector.memset(warm_sb[:], 0.0)

                # warm-up matmuls: keep the PE busy during the input DMA so
                # the HAM clock gate opens before real work arrives.
                wm = PS.tile([128, FW], F32, tag="qk", bufs=2)
                for r in range(10):
                    nc.tensor.matmul(
                        wm[:], lhsT=warm_sb[:, 0:128], rhs=warm_sb[:],
                        start=(r == 0), stop=(r == 9),
                    )

                # ---- QKV emission helpers ----
                def emit_qk(dst, blk, fs_list):
                    for f in fs_list:
                        ps = PS.tile([128, FW], F32, tag="qk", bufs=2)
                        for ct in range(CT):
                            nc.tensor.matmul(
                                ps[:],
                                lhsT=wqk_sb[:, ct,
                                            blk * 128:(blk + 1) * 128],
                                rhs=xT_sb[:, ct, f * FW:(f + 1) * FW],
                                start=(ct == 0), stop=(ct == CT - 1),
                            )
                            del ct
                        nc.vector.tensor_copy(
                            dst[:, f * FW:(f + 1) * FW], ps[:])

                def emit_v():
                    for nt in range(NT):
                        v_ps = PS.tile([128, FW], F32, tag="qk", bufs=2)
                        for ct in range(CT):
                            nc.tensor.matmul(
                                v_ps[:, 0:CHL],
                                lhsT=xT_sb[:, ct, nt * 128:(nt + 1) * 128],
                                rhs=wv_sb[:, ct, :],
                                start=(ct == 0), stop=(ct == CT - 1),
                            )
                        nc.vector.tensor_copy(
                            v_sb[:, nt, :, HD:2 * HD],
                            v_ps[:, 0:CHL].rearrange("p (h d) -> p h d",
                                                     h=HL))

                heads = ((q01_sb[0:64], k01_sb[0:64]),
                        (q01_sb[64:128], k01_sb[64:128]),
                        (qk2_sb[0:64], k2_sb[0:64]))

                def emit_head(w, h):
                    q0 = w * QW
                    qh, kh = heads[h]
                    A = PS.tile([128, QW], F32, tag="A", bufs=1)
                    # software-pipelined: emit S(kc+1) before AV(kc)
                    pend = []
                    for kc in range(KT):
                        S = PS.tile([128, QW], F32, tag="S", bufs=2)
                        E = AS.tile([128, QW], BF16, tag="E", bufs=4)
                        for j in range(QW // FW):
                            js = slice(j * FW, (j + 1) * FW)
                            nc.tensor.matmul(
                                S[:, js],
                                lhsT=kh[:, kc * 128:(kc + 1) * 128],
                                rhs=qh[:, q0 + j * FW:q0 + (j + 1) * FW],
                            )
                        nc.scalar.activation(E[:], S[:], AF.Exp, scale=SCALE)
                        pend.append((kc, E))
                        if len(pend) == 2:
                            _emit_av(A, h, *pend.pop(0), last=False)
                    _emit_av(A, h, *pend.pop(0), last=True)
                    # normalize: free A quickly by copying both live regions
                    # to SBUF, then run the whole chain off the critical path.
                    aout = AS.tile([128, QW], F32, tag="aout", bufs=2)
                    nc.vector.tensor_copy(aout[64:128, :], A[64:128, :])
                    nc.vector.tensor_copy(aout[0:1, :], A[0:1, :])
                    dT = AS.tile([64, QW // 64], F32, tag="dT", bufs=2)
                    rT = AS.tile([64, QW // 64], F32, tag="rT", bufs=2)
                    r0 = AS.tile([1, QW], F32, tag="r0", bufs=2)
                    bcs = AS.tile([128, QW], F32, tag="bcs", bufs=2)
                    attn_t = AS.tile([128, QW], BF16, tag="attn", bufs=3)
                    nc.sync.dma_start(out=dT[:], in_=aout[0:1, :])
                    nc.vector.reciprocal(rT[:], dT[:])
                    nc.sync.dma_start(out=r0[:], in_=rT[:])
                    nc.gpsimd.partition_broadcast(bcs[:], r0[0:1, :])
                    nc.vector.tensor_mul(attn_t[64:128, :], aout[64:128, :],
                                         bcs[64:128, :])
                    nc.sync.dma_start(
                        out=ag_ins[w][h * HD:(h + 1) * HD, :],
                        in_=attn_t[64:128, :],
                    )

                def _emit_av(A, h, kc, E, last):
                    for j in range(QW // FW):
                        js = slice(j * FW, (j + 1) * FW)
                        nc.tensor.matmul(
                            A[:, js],
                            lhsT=v_sb[:, kc, h, :],
                            rhs=E[:, js],
                            start=(kc == 0), stop=last,
                        )

                def emit_gather(w):
                    nc.gpsimd.collective_compute(
                        "AllGather",
                        mybir.AluOpType.bypass,
                        replica_groups=[[0, 1, 2, 3], [4, 5, 6, 7]],
                        ins=[ag_ins[w].opt()],
                        outs=[ag_outs[w].opt()],
                    )

                # ---- emission schedule ----
                emit_qk(k01_sb, 1, [0, 1, 2, 3])
                emit_qk(q01_sb, 0, [0, 1])
                emit_v()
                emit_head(0, 0)
                emit_qk(qk2_sb, 2, [0, 1, 2, 3])
                emit_qk(q01_sb, 0, [2, 3])
                # k2 shard -> its own partition-0-based tile
                nc.sync.dma_start(out=k2_sb[:], in_=qk2_sb[64:128, :])
                emit_head(0, 1)
                emit_head(0, 2)
                emit_gather(0)
                for h in range(HL):
                    emit_head(1, h)
                emit_gather(1)
                QIN.release()

                # ---- output projection (out^T [CHL, N]) ----
                for f in range(N // FW):
                    wf, jf = divmod(f, QW // FW)
                    ao = AS.tile([128, KP, FW], BF16, tag="ao", bufs=2)
                    for kp in range(KP):
                        nc.sync.dma_start(
                            out=ao[:, kp, :],
                            in_=ag_outs[wf][kp * 128:(kp + 1) * 128,
                                            jf * FW:(jf + 1) * FW],
                        )
                    for mlo, mhi, bp_sb in ((0, 128, bpa_sb),
                                            (128, CHL, bpb_sb)):
                        m = mhi - mlo
                        pr = PS.tile([128, FW], F32, tag="qk", bufs=2)
                        for kp in range(KP):
                            nc.tensor.matmul(
                                pr[0:m, :],
                                lhsT=wp_sb[:, kp, mlo:mhi],
                                rhs=ao[:, kp, :],
                                start=(kp == 0), stop=(kp == KP - 1),
                            )
                        o_t = AS.tile([128, FW], F32, tag="o", bufs=3)
                        nc.vector.tensor_scalar_add(o_t[0:m, :], pr[0:m, :],
                                                    bp_sb[0:m])
                        nc.sync.dma_start(
                            out=out_d[mlo:mhi, f * FW:(f + 1) * FW],
                            in_=o_t[0:m, :],
                        )
    nc.finalize()
    return nc


def get_nc():
    if "nc" not in _CACHE:
        _CACHE["nc"] = _build_nc()
    return _CACHE["nc"]


def make_in_maps(x, w_qkv, w_proj, b_proj):
    import ml_dtypes

    BF = ml_dtypes.bfloat16
    x = np.asarray(x, dtype=np.float32)
    w_qkv = np.asarray(w_qkv, dtype=np.float32)
    w_proj = np.asarray(w_proj, dtype=np.float32)
    b_proj = np.asarray(b_proj, dtype=np.float32)
    in_maps = []
    for core in range(NCORES):
        b, g = divmod(core, G)
        cs = slice(g * CHL, (g + 1) * CHL)
        wq = w_qkv[:, 0 * C:1 * C][:, cs]
        wk = w_qkv[:, 1 * C:2 * C][:, cs]
        wqk = np.concatenate(
            [wq[:, 0:128], wk[:, 0:128], wq[:, 128:CHL], wk[:, 128:CHL]],
            axis=1)
        im = {
            "xT": np.ascontiguousarray(x[b].T).astype(BF),
            "wqk": np.ascontiguousarray(wqk).astype(BF),
            "wv": np.ascontiguousarray(
                w_qkv[:, 2 * C:3 * C][:, cs]).astype(BF),
            "wp": np.ascontiguousarray(w_proj[:, cs]).astype(BF),
            "bp": np.ascontiguousarray(b_proj[cs].reshape(CHL, 1)),
        }
        in_maps.append(im)
    return in_maps


def unshard(results):
    out = np.empty((B, N, C), dtype=np.float32)
    for b in range(B):
        outT = np.concatenate(
            [results[b * G + g]["out"] for g in range(G)], axis=0)
        out[b] = outT.T
    return out


def kernel(x, w_qkv, w_proj, b_proj):
    from concourse.bass_utils import run_bass_kernel_spmd

    nc = get_nc()
    in_maps = make_in_maps(x, w_qkv, w_proj, b_proj)
    res = run_bass_kernel_spmd(nc, in_maps, list(range(NCORES)))
    return unshard(res.results)
